# revision 13
# baseline (speedup 1.0000x reference)
"""BitPackedLinear Trainium2 kernel (8-core SPMD, token-sharded, fp8 DoubleRow).

y = x @ W.T + bias, W = unpack_bits(packed_weight) in {-1,+1}, shapes:
  x [2, 2048, 4096] f32, packed_weight [4096, 512] u8, bias [4096] f32.

Sharding: data-parallel over tokens (4096 tokens -> 512/core). Each core
computes y_c = x_c @ W.T + bias for its token shard against the full
weight; the host just concatenates shards.

Device algorithm per core (matmuls are fp8e4 DoubleRow at 0.5 cyc/row,
one instruction contracts TWO 128-deep k-tiles):
  - x is split as x_bf = hi + lo with hi = e4m3(x_bf), lo = x_bf - hi
    (lo is exactly representable in e4m3, so hi+lo == bf16(x) exactly;
    end-to-end rel err ~1.7e-3, dominated by the bf16 load cast).
  - i-tiling: i = 2048h + 16j + 8p + b with j the PE partition, (p, b)
    byte-parity/bit, h the 2048-halves. The DoubleRow k-tile pair is p.
  - weights: pw is viewed as u16 [4096, 256] (byte pairs along k) and
    XBAR-transpose-DMA'd to byteT2 [128j, h, o]; each u16 holds bytes
    (2j, 2j+1) of one o. A single u16 shift+mask (<<(6-b), & 0x4040)
    then yields the e4m3 BIT PATTERN of {0, 2.0} (0x40) for BOTH
    parities of bit b at once -> one DVE op per (h, b, o-slab), all in
    the DVE 4x perf mode. The u16 result bitcast to fp8 is the moving
    [j, p, o] operand.
  - x chunks arrive as f32->bf16 SWDGE cast-DMAs [128t, 2048i]; PE
    transposes (1 cyc/row) produce [j, t] bf16 tiles grouped in wide
    PSUM tiles; ACT casts psum->fp8 (hi), DVE subtracts psum - hi -> lo
    (mixed-dtype in, fp8 out).
  - psum[t, o] accumulates 32 DoubleRow matmuls (2 passes x 2 h x 8 b).
  - bias is pre-broadcast via a rank-1 f32r matmul into bbc; rowsum
    s[t] = sum_i bf16(x)[t, i] on Pool (consistent with hi+lo);
    epilogue fuses y = psum - s + bbc on DVE/Pool.
"""
import sys

sys.path.insert(0, "/opt/trn_rl_repo")
from contextlib import ExitStack

import numpy as np

import concourse.tile as tile
from concourse import bacc, mybir
from concourse.bass import ts
from concourse.bass_utils import run_bass_kernel_spmd
from concourse.masks import make_identity

F32 = mybir.dt.float32
F32R = mybir.dt.float32r
BF16 = mybir.dt.bfloat16
U16 = mybir.dt.uint16
F8 = mybir.dt.float8e4
P = 128

N_CORES = 8
B_DIM, S_DIM, I_DIM, O_DIM = 2, 2048, 4096, 4096
T_FULL = B_DIM * S_DIM          # 4096 tokens
T_SHARD = T_FULL // N_CORES     # 512 tokens per core
OUT_NAME = "y"


def build(T=T_SHARD, I=I_DIM, O=O_DIM, n_cores=N_CORES):
    H = I // 2048               # 2048-wide i-halves (j spans 16*128)
    TT = T // P                 # token tiles
    NB = 8                      # bits per byte
    K2 = I // 16                # u16 byte-pairs per weight row
    OSL = 512                   # o-slab width
    NSL = O // OSL

    nc = bacc.Bacc("TRN2", target_bir_lowering=False, debug=False,
                   num_devices=n_cores)
    x_d = nc.dram_tensor("x", [T, I], F32, kind="ExternalInput").ap()
    pw16_d = nc.dram_tensor("pw16", [O, K2], U16, kind="ExternalInput").ap()
    bias_d = nc.dram_tensor("bias", [O], F32, kind="ExternalInput").ap()
    y_d = nc.dram_tensor(OUT_NAME, [T, O], F32, kind="ExternalOutput").ap()

    with tile.TileContext(nc) as tc:
        with ExitStack() as ctx:
            const = ctx.enter_context(tc.tile_pool(name="const", bufs=1))
            persist = ctx.enter_context(tc.tile_pool(name="persist", bufs=1))
            stage = ctx.enter_context(tc.tile_pool(name="stage", bufs=1))

            ident_bf = const.tile([P, P], BF16)
            make_identity(nc, ident_bf[:])
            ones_r = const.tile([1, P], F32R)
            bias_r = const.tile([1, O], F32R)

            byteT2 = persist.tile([P, H, O], U16)
            # xT planes: [j, h, bh, tt, (b' p t)] fp8, 16KB/partition each
            xT_hi = persist.tile([P, H, 2, TT, 1024], F8)
            xT_lo = persist.tile([P, H, 2, TT, 1024], F8)

            xn_pool = ctx.enter_context(tc.tile_pool(name="xn", bufs=H * TT))
            wt_pool = ctx.enter_context(tc.tile_pool(name="wt", bufs=2))
            scol_pool = ctx.enter_context(tc.tile_pool(name="scol", bufs=1))
            trash_pool = ctx.enter_context(tc.tile_pool(name="trash", bufs=2))
            bbc_pool = ctx.enter_context(tc.tile_pool(name="bbc", bufs=2))
            y_pool = ctx.enter_context(tc.tile_pool(name="ysb", bufs=4))
            ps_tr = ctx.enter_context(
                tc.tile_pool(name="ps_tr", bufs=3, space="PSUM"))
            ps_mm = ctx.enter_context(
                tc.tile_pool(name="ps_mm", bufs=4, space="PSUM"))
            ps_b_pool = ctx.enter_context(
                tc.tile_pool(name="ps_b", bufs=1, space="PSUM"))

            # --- weight bytes: XBAR transpose-DMAs (split for latency) ---
            for oh in range(2):
                for h in range(H):
                    nc.sync.dma_start_transpose(
                        byteT2[:, h, ts(oh, O // 2)],
                        pw16_d[ts(oh, O // 2), ts(h, P)])

            # --- x chunks: SWDGE f32->bf16 cast DMAs, tt-major ---
            xns = {}
            for tt in range(TT):
                for h in range(H):
                    xn = xn_pool.tile([P, P, 16], BF16, tag="xn")
                    nc.gpsimd.dma_start(
                        xn[:].rearrange("t j q -> t (j q)"),
                        x_d[ts(tt, P), ts(h, 2048)],
                    )
                    xns[h, tt] = xn

            # --- bias/ones staging ---
            ones_f32 = stage.tile([1, P], F32)
            nc.vector.memset(ones_f32[:], 1.0)
            nc.vector.tensor_copy(out=ones_r[:], in_=ones_f32[:])
            bias_f32 = stage.tile([1, O], F32)
            nc.sync.dma_start(
                bias_f32[:], bias_d.rearrange("(b o) -> b o", b=1))
            nc.vector.tensor_copy(out=bias_r[:], in_=bias_f32[:])

            # --- unpack weight slabs 0,1 up front (DVE, 4x mode) ---
            def unpack(sl, wt):
                for h in range(H):
                    for b in range(NB):
                        if b < 7:
                            nc.vector.tensor_scalar(
                                out=wt[:, h, b, :],
                                in0=byteT2[:, h, ts(sl, OSL)],
                                scalar1=6 - b, scalar2=0x4040,
                                op0=mybir.AluOpType.logical_shift_left,
                                op1=mybir.AluOpType.bitwise_and,
                            )
                        else:
                            nc.vector.tensor_scalar(
                                out=wt[:, h, b, :],
                                in0=byteT2[:, h, ts(sl, OSL)],
                                scalar1=1, scalar2=0x4040,
                                op0=mybir.AluOpType.logical_shift_right,
                                op1=mybir.AluOpType.bitwise_and,
                            )

            wts = {}
            for sl in range(min(2, NSL)):
                wts[sl] = wt_pool.tile([P, H, NB, OSL], U16, name="wt", tag="wt")
                unpack(sl, wts[sl])

            def mm_psum(sl, tt, wt, bbc):
                ps = ps_mm.tile([P, OSL], F32, name="ps", tag="mm")
                n = 0
                for plane in (xT_hi, xT_lo):
                    for h in range(H):
                        for bh in range(2):
                            for bp in range(4):
                                rhs = wt[:, h, 4 * bh + bp, :].bitcast(
                                    F8).rearrange("j (o p) -> j p o", p=2)
                                nc.tensor.matmul(
                                    ps[:],
                                    plane[:, h, bh, tt, ts(bp, 256)]
                                    .rearrange("j (p t) -> j p t", p=2),
                                    rhs,
                                    start=(n == 0), stop=(n == 31),
                                    perf_mode=mybir.MatmulPerfMode.DoubleRow,
                                )
                                n += 1
                y_sb = y_pool.tile([P, OSL], F32, name="y_sb", tag="y")
                nc.vector.scalar_tensor_tensor(
                    out=y_sb[:], in0=ps[:], scalar=s_col[:, tt:tt + 1],
                    in1=bbc[:],
                    op0=mybir.AluOpType.subtract,
                    op1=mybir.AluOpType.add,
                )
                nc.sync.dma_start(y_d[ts(tt, P), ts(sl, OSL)], y_sb[:])

            # bias broadcast for slab 0 (needed by the interleaved sl0 psums)
            ps_bias0 = ps_b_pool.tile([P, OSL], F32, name="psb", tag="psb")
            nc.tensor.matmul(ps_bias0[:], ones_r[:], bias_r[:, ts(0, OSL)],
                             start=True, stop=True)
            bbc0 = bbc_pool.tile([P, OSL], F32, name="bbc", tag="bbc")
            nc.scalar.copy(out=bbc0[:], in_=ps_bias0[:])

            # --- transposes + hi/lo split + rowsum, chunk-arrival order,
            # --- with sl0's matmuls interleaved per token tile
            parts = scol_pool.tile([P, TT, H], F32)
            s_col = scol_pool.tile([P, TT], F32)
            for tt in range(TT):
                for h in range(H):
                    for bh in range(2):
                        ps = ps_tr.tile([P, 1024], BF16, tag="tr")
                        for bp in range(4):
                            for p in range(2):
                                nc.tensor.transpose(
                                    ps[:, ts(bp * 2 + p, P)],
                                    xns[h, tt][:, :, 8 * p + 4 * bh + bp],
                                    ident_bf[:],
                                )
                        nc.scalar.copy(out=xT_hi[:, h, bh, tt, :], in_=ps[:])
                        nc.vector.tensor_tensor(
                            out=xT_lo[:, h, bh, tt, :], in0=ps[:],
                            in1=xT_hi[:, h, bh, tt, :],
                            op=mybir.AluOpType.subtract,
                        )
                    # rowsum partial via tensor_scalar accumulator (2x mode)
                    trash = trash_pool.tile([P, P, 16], BF16, tag="trash")
                    nc.vector.tensor_scalar(
                        out=trash[:], in0=xns[h, tt][:],
                        scalar1=1.0, scalar2=0.0,
                        op0=mybir.AluOpType.mult, op1=mybir.AluOpType.add,
                        accum_out=parts[:, tt, h:h + 1],
                    )
                    if h == H - 1:
                        nc.vector.tensor_tensor(
                            out=s_col[:, tt:tt + 1], in0=parts[:, tt, 0:1],
                            in1=parts[:, tt, 1:2], op=mybir.AluOpType.add,
                        )
                mm_psum(0, tt, wts[0], bbc0)

            # --- main loop over remaining o-slabs ---
            wts.pop(0)
            if NSL > 2:
                wts[2] = wt_pool.tile([P, H, NB, OSL], U16, name="wt", tag="wt")
                unpack(2, wts[2])
            for sl in range(1, NSL):
                wt = wts.pop(sl)
                ps_bias = ps_b_pool.tile([P, OSL], F32, name="psb", tag="psb")
                nc.tensor.matmul(
                    ps_bias[:], ones_r[:], bias_r[:, ts(sl, OSL)],
                    start=True, stop=True,
                )
                bbc = bbc_pool.tile([P, OSL], F32, name="bbc", tag="bbc")
                nc.scalar.copy(out=bbc[:], in_=ps_bias[:])

                for tt in range(TT):
                    mm_psum(sl, tt, wt, bbc)

                if sl + 2 < NSL:
                    wts[sl + 2] = wt_pool.tile([P, H, NB, OSL], U16, name="wt", tag="wt")
                    unpack(sl + 2, wts[sl + 2])

    nc.compile()
    return nc


_NC = None


def _get_nc():
    global _NC
    if _NC is None:
        _NC = build()
    return _NC


def run(x, packed_weight, bias, trace=False):
    x = np.ascontiguousarray(np.asarray(x, dtype=np.float32))
    pw = np.ascontiguousarray(np.asarray(packed_weight).astype(np.uint8))
    bias = np.ascontiguousarray(np.asarray(bias, dtype=np.float32))
    assert x.shape == (B_DIM, S_DIM, I_DIM)
    assert pw.shape == (O_DIM, I_DIM // 8)
    assert bias.shape == (O_DIM,)

    nc = _get_nc()
    xs = x.reshape(T_FULL, I_DIM)
    pw16 = pw.view(np.uint16)
    in_maps = [
        {
            "x": np.ascontiguousarray(xs[c * T_SHARD:(c + 1) * T_SHARD]),
            "pw16": pw16,
            "bias": bias,
        }
        for c in range(N_CORES)
    ]
    res = run_bass_kernel_spmd(nc, in_maps, list(range(N_CORES)), trace=trace)
    y = np.concatenate(
        [res.results[c][OUT_NAME] for c in range(N_CORES)], axis=0
    )
    return y.reshape(B_DIM, S_DIM, O_DIM), res


def kernel(x, packed_weight, bias):
    y, _ = run(x, packed_weight, bias, trace=False)
    return y


# revision 20
# speedup vs baseline: 1.0289x; 1.0289x over previous
"""BitPackedLinear Trainium2 kernel (8-core SPMD, token-sharded, fp8 DoubleRow).

y = x @ W.T + bias, W = unpack_bits(packed_weight) in {-1,+1}, shapes:
  x [2, 2048, 4096] f32, packed_weight [4096, 512] u8, bias [4096] f32.

Sharding: data-parallel over tokens (4096 tokens -> 512/core). Each core
computes y_c = x_c @ W.T + bias for its token shard against the full
weight; the host just concatenates shards.

Device algorithm per core (matmuls are fp8e4 DoubleRow at 0.5 cyc/row,
one instruction contracts TWO 128-deep k-tiles):
  - x is split as x_bf = hi + lo with hi = e4m3(x_bf), lo = x_bf - hi
    (lo is exactly representable in e4m3, so hi+lo == bf16(x) exactly;
    end-to-end rel err ~1.7e-3, dominated by the bf16 load cast).
  - i-tiling: i = 2048h + 16j + 8p + b with j the PE partition, (p, b)
    byte-parity/bit, h the 2048-halves. The DoubleRow k-tile pair is p.
  - weights: pw is viewed as u16 [4096, 256] (byte pairs along k) and
    XBAR-transpose-DMA'd to byteT2 [128j, h, o]; each u16 holds bytes
    (2j, 2j+1) of one o. A single u16 shift+mask (<<(6-b), & 0x4040)
    then yields the e4m3 BIT PATTERN of {0, 2.0} (0x40) for BOTH
    parities of bit b at once -> one DVE op per (h, b, o-slab), all in
    the DVE 4x perf mode. The u16 result bitcast to fp8 is the moving
    [j, p, o] operand.
  - x chunks arrive as f32->bf16 SWDGE cast-DMAs [128t, 2048i]; PE
    transposes (1 cyc/row) produce [j, t] bf16 tiles grouped in wide
    PSUM tiles; ACT casts psum->fp8 (hi), DVE subtracts psum - hi -> lo
    (mixed-dtype in, fp8 out).
  - psum[t, o] accumulates 32 DoubleRow matmuls (2 passes x 2 h x 8 b).
  - bias is pre-broadcast via a rank-1 f32r matmul into bbc; rowsum
    s[t] = sum_i bf16(x)[t, i] on Pool (consistent with hi+lo);
    epilogue fuses y = psum - s + bbc on DVE/Pool.
"""
import sys

sys.path.insert(0, "/opt/trn_rl_repo")
from contextlib import ExitStack

import numpy as np

import concourse.tile as tile
from concourse import bacc, mybir
from concourse.bass import ts
from concourse.bass_utils import run_bass_kernel_spmd
from concourse.masks import make_identity

F32 = mybir.dt.float32
F32R = mybir.dt.float32r
BF16 = mybir.dt.bfloat16
U16 = mybir.dt.uint16
F8 = mybir.dt.float8e4
P = 128

N_CORES = 8
B_DIM, S_DIM, I_DIM, O_DIM = 2, 2048, 4096, 4096
T_FULL = B_DIM * S_DIM          # 4096 tokens
T_SHARD = T_FULL // N_CORES     # 512 tokens per core
OUT_NAME = "y"


def build(T=T_SHARD, I=I_DIM, O=O_DIM, n_cores=N_CORES):
    H = I // 2048               # 2048-wide i-halves (j spans 16*128)
    TT = T // P                 # token tiles
    NB = 8                      # bits per byte
    K2 = I // 16                # u16 byte-pairs per weight row
    OSL = 512                   # o-slab width
    NSL = O // OSL

    nc = bacc.Bacc("TRN2", target_bir_lowering=False, debug=False,
                   num_devices=n_cores)
    x_d = nc.dram_tensor("x", [T, I], F32, kind="ExternalInput").ap()
    pw16_d = nc.dram_tensor("pw16", [O, K2], U16, kind="ExternalInput").ap()
    bias_d = nc.dram_tensor("bias", [O], F32, kind="ExternalInput").ap()
    y_d = nc.dram_tensor(OUT_NAME, [T, O], F32, kind="ExternalOutput").ap()

    with tile.TileContext(nc) as tc:
        with ExitStack() as ctx:
            const = ctx.enter_context(tc.tile_pool(name="const", bufs=1))
            persist = ctx.enter_context(tc.tile_pool(name="persist", bufs=1))
            stage = ctx.enter_context(tc.tile_pool(name="stage", bufs=1))

            ident_bf = const.tile([P, P], BF16)
            make_identity(nc, ident_bf[:])
            ones_r = const.tile([1, P], F32R)
            bias_r = const.tile([1, O], F32R)

            byteT2 = persist.tile([P, H, O], U16)
            # xT planes: [j, h, bh, tt, (b' p t)] fp8, 16KB/partition each
            xT_hi = persist.tile([P, H, 2, TT, 1024], F8)
            xT_lo = persist.tile([P, H, 2, TT, 1024], F8)

            xn_pool = ctx.enter_context(tc.tile_pool(name="xn", bufs=H * TT))
            pk_pool = ctx.enter_context(tc.tile_pool(name="pk", bufs=2))
            wt_pool = ctx.enter_context(tc.tile_pool(name="wt", bufs=2))
            scol_pool = ctx.enter_context(tc.tile_pool(name="scol", bufs=1))
            trash_pool = ctx.enter_context(tc.tile_pool(name="trash", bufs=2))
            bbc_pool = ctx.enter_context(tc.tile_pool(name="bbc", bufs=2))
            y_pool = ctx.enter_context(tc.tile_pool(name="ysb", bufs=4))
            ps_tr = ctx.enter_context(
                tc.tile_pool(name="ps_tr", bufs=2, space="PSUM"))
            ps_btr = ctx.enter_context(
                tc.tile_pool(name="ps_btr", bufs=2, space="PSUM"))
            ps_mm = ctx.enter_context(
                tc.tile_pool(name="ps_mm", bufs=3, space="PSUM"))
            ps_b_pool = ctx.enter_context(
                tc.tile_pool(name="ps_b", bufs=1, space="PSUM"))

            # pw quarters: 8 o-tiles (1024 o's) per regular DMA, u16 pairs
            OT = O // P
            QOT = min(8, OT)            # o-tiles per quarter
            NQ = OT // QOT
            pw_ap = pw16_d.rearrange("(ot p) k -> p ot k", p=P)
            pks = {}

            def pw_dma(q):
                pk = pk_pool.tile([P, QOT, K2], U16, name="pk", tag="pk")
                nc.gpsimd.dma_start(pk[:], pw_ap[:, ts(q, QOT), :])
                pks[q] = pk

            # PE u16 transposes of byte pairs + DVE evac into byteT2
            def pw_tr_evac(q):
                pk = pks.pop(q)
                for h in range(H):
                    for g in range(QOT // 4):
                        btr = ps_btr.tile([P, 4, P], BF16, name="btr", tag="btr")
                        for otl in range(4):
                            nc.tensor.transpose(
                                btr[:, otl, :],
                                pk[:, 4 * g + otl, ts(h, P)].bitcast(BF16),
                                ident_bf[:],
                            )
                        nc.vector.tensor_copy(
                            out=byteT2[:, h, q * QOT * P + 512 * g:
                                       q * QOT * P + 512 * (g + 1)],
                            in_=btr[:].rearrange("j a b -> j (a b)").bitcast(U16),
                        )

            # --- x chunks (SWDGE f32->bf16 cast DMAs) interleaved with pw ---
            xns = {}
            for tt in range(TT):
                for h in range(H):
                    xn = xn_pool.tile([P, P, 16], BF16, tag="xn")
                    nc.gpsimd.dma_start(
                        xn[:].rearrange("t j q -> t (j q)"),
                        x_d[ts(tt, P), ts(h, 2048)],
                    )
                    xns[h, tt] = xn
                if tt < NQ:
                    pw_dma(tt)
            for q in range(TT, NQ):
                pw_dma(q)

            # --- bias/ones staging ---
            ones_f32 = stage.tile([1, P], F32)
            nc.vector.memset(ones_f32[:], 1.0)
            nc.vector.tensor_copy(out=ones_r[:], in_=ones_f32[:])
            bias_f32 = stage.tile([1, O], F32)
            nc.sync.dma_start(
                bias_f32[:], bias_d.rearrange("(b o) -> b o", b=1))
            nc.vector.tensor_copy(out=bias_r[:], in_=bias_f32[:])

            # --- unpack weight slabs 0,1 up front (DVE, 4x mode) ---
            def unpack(sl, wt):
                for h in range(H):
                    for b in range(NB):
                        if b < 7:
                            nc.vector.tensor_scalar(
                                out=wt[:, h, b, :],
                                in0=byteT2[:, h, ts(sl, OSL)],
                                scalar1=6 - b, scalar2=0x4040,
                                op0=mybir.AluOpType.logical_shift_left,
                                op1=mybir.AluOpType.bitwise_and,
                            )
                        else:
                            nc.vector.tensor_scalar(
                                out=wt[:, h, b, :],
                                in0=byteT2[:, h, ts(sl, OSL)],
                                scalar1=1, scalar2=0x4040,
                                op0=mybir.AluOpType.logical_shift_right,
                                op1=mybir.AluOpType.bitwise_and,
                            )

            wts = {}

            def alloc_unpack(sl):
                wts[sl] = wt_pool.tile([P, H, NB, OSL], U16, name="wt",
                                       tag="wt")
                unpack(sl, wts[sl])

            def mm_psum(sl, tt, wt, bbc):
                ps = ps_mm.tile([P, OSL], F32, name="ps", tag="mm")
                n = 0
                for plane in (xT_hi, xT_lo):
                    for h in range(H):
                        for bh in range(2):
                            for bp in range(4):
                                rhs = wt[:, h, 4 * bh + bp, :].bitcast(
                                    F8).rearrange("j (o p) -> j p o", p=2)
                                nc.tensor.matmul(
                                    ps[:],
                                    plane[:, h, bh, tt, ts(bp, 256)]
                                    .rearrange("j (p t) -> j p t", p=2),
                                    rhs,
                                    start=(n == 0), stop=(n == 31),
                                    perf_mode=mybir.MatmulPerfMode.DoubleRow,
                                )
                                n += 1
                y_sb = y_pool.tile([P, OSL], F32, name="y_sb", tag="y")
                nc.vector.scalar_tensor_tensor(
                    out=y_sb[:], in0=ps[:], scalar=s_col[:, tt:tt + 1],
                    in1=bbc[:],
                    op0=mybir.AluOpType.subtract,
                    op1=mybir.AluOpType.add,
                )
                nc.sync.dma_start(y_d[ts(tt, P), ts(sl, OSL)], y_sb[:])

            # bias broadcast for slab 0 (needed by the interleaved sl0 psums)
            ps_bias0 = ps_b_pool.tile([P, OSL], F32, name="psb", tag="psb")
            nc.tensor.matmul(ps_bias0[:], ones_r[:], bias_r[:, ts(0, OSL)],
                             start=True, stop=True)
            bbc0 = bbc_pool.tile([P, OSL], F32, name="bbc", tag="bbc")
            nc.scalar.copy(out=bbc0[:], in_=ps_bias0[:])

            def splits(tt):
                for h in range(H):
                    for bh in range(2):
                        ps = ps_tr.tile([P, 1024], BF16, tag="tr")
                        for bp in range(4):
                            for p in range(2):
                                nc.tensor.transpose(
                                    ps[:, ts(bp * 2 + p, P)],
                                    xns[h, tt][:, :, 8 * p + 4 * bh + bp],
                                    ident_bf[:],
                                )
                        nc.scalar.copy(out=xT_hi[:, h, bh, tt, :], in_=ps[:])
                        nc.vector.tensor_tensor(
                            out=xT_lo[:, h, bh, tt, :], in0=ps[:],
                            in1=xT_hi[:, h, bh, tt, :],
                            op=mybir.AluOpType.subtract,
                        )
                    # rowsum partial via tensor_scalar accumulator (on Pool)
                    trash = trash_pool.tile([P, P, 16], BF16, tag="trash")
                    nc.vector.tensor_scalar(
                        out=trash[:], in0=xns[h, tt][:],
                        scalar1=1.0, scalar2=0.0,
                        op0=mybir.AluOpType.mult, op1=mybir.AluOpType.add,
                        accum_out=parts[:, tt, h:h + 1],
                    )
                    if h == H - 1:
                        nc.vector.tensor_tensor(
                            out=s_col[:, tt:tt + 1], in0=parts[:, tt, 0:1],
                            in1=parts[:, tt, 1:2], op=mybir.AluOpType.add,
                        )

            # --- startup: splits, pw transposes, first unpacks, and sl0's
            # --- matmuls interleaved per token tile
            parts = scol_pool.tile([P, TT, H], F32)
            s_col = scol_pool.tile([P, TT], F32)
            for tt in range(TT):
                splits(tt)
                if tt < NQ:
                    pw_tr_evac(tt)
                if tt == 0:
                    alloc_unpack(0)
                elif tt == 1 and NSL > 1:
                    alloc_unpack(1)
                mm_psum(0, tt, wts[0], bbc0)
            for q in range(TT, NQ):
                pw_tr_evac(q)
            if NSL > 1 and 1 not in wts:
                alloc_unpack(1)

            # --- main loop over remaining o-slabs ---
            wts.pop(0)
            if NSL > 2:
                alloc_unpack(2)
            for sl in range(1, NSL):
                wt = wts.pop(sl)
                ps_bias = ps_b_pool.tile([P, OSL], F32, name="psb", tag="psb")
                nc.tensor.matmul(
                    ps_bias[:], ones_r[:], bias_r[:, ts(sl, OSL)],
                    start=True, stop=True,
                )
                bbc = bbc_pool.tile([P, OSL], F32, name="bbc", tag="bbc")
                nc.scalar.copy(out=bbc[:], in_=ps_bias[:])

                for tt in range(TT):
                    mm_psum(sl, tt, wt, bbc)

                if sl + 2 < NSL:
                    alloc_unpack(sl + 2)

    nc.compile()
    return nc


_NC = None


def _get_nc():
    global _NC
    if _NC is None:
        _NC = build()
    return _NC


def run(x, packed_weight, bias, trace=False):
    x = np.ascontiguousarray(np.asarray(x, dtype=np.float32))
    pw = np.ascontiguousarray(np.asarray(packed_weight).astype(np.uint8))
    bias = np.ascontiguousarray(np.asarray(bias, dtype=np.float32))
    assert x.shape == (B_DIM, S_DIM, I_DIM)
    assert pw.shape == (O_DIM, I_DIM // 8)
    assert bias.shape == (O_DIM,)

    nc = _get_nc()
    xs = x.reshape(T_FULL, I_DIM)
    pw16 = pw.view(np.uint16)
    in_maps = [
        {
            "x": np.ascontiguousarray(xs[c * T_SHARD:(c + 1) * T_SHARD]),
            "pw16": pw16,
            "bias": bias,
        }
        for c in range(N_CORES)
    ]
    res = run_bass_kernel_spmd(nc, in_maps, list(range(N_CORES)), trace=trace)
    y = np.concatenate(
        [res.results[c][OUT_NAME] for c in range(N_CORES)], axis=0
    )
    return y.reshape(B_DIM, S_DIM, O_DIM), res


def kernel(x, packed_weight, bias):
    y, _ = run(x, packed_weight, bias, trace=False)
    return y


# revision 22
# speedup vs baseline: 1.0347x; 1.0056x over previous
"""BitPackedLinear Trainium2 kernel (8-core SPMD, token-sharded, fp8 DoubleRow).

y = x @ W.T + bias, W = unpack_bits(packed_weight) in {-1,+1}, shapes:
  x [2, 2048, 4096] f32, packed_weight [4096, 512] u8, bias [4096] f32.

Sharding: data-parallel over tokens (4096 tokens -> 512/core). Each core
computes y_c = x_c @ W.T + bias for its token shard against the full
weight; the host just concatenates shards.

Device algorithm per core (matmuls are fp8e4 DoubleRow at 0.5 cyc/row,
one instruction contracts TWO 128-deep k-tiles):
  - x is split as x_bf = hi + lo with hi = e4m3(x_bf), lo = x_bf - hi
    (lo is exactly representable in e4m3, so hi+lo == bf16(x) exactly;
    end-to-end rel err ~1.7e-3, dominated by the bf16 load cast).
  - i-tiling: i = 2048h + 16j + 8p + b with j the PE partition, (p, b)
    byte-parity/bit, h the 2048-halves. The DoubleRow k-tile pair is p.
  - weights: pw is viewed as u16 [4096, 256] (byte pairs along k) and
    XBAR-transpose-DMA'd to byteT2 [128j, h, o]; each u16 holds bytes
    (2j, 2j+1) of one o. A single u16 shift+mask (<<(6-b), & 0x4040)
    then yields the e4m3 BIT PATTERN of {0, 2.0} (0x40) for BOTH
    parities of bit b at once -> one DVE op per (h, b, o-slab), all in
    the DVE 4x perf mode. The u16 result bitcast to fp8 is the moving
    [j, p, o] operand.
  - x chunks arrive as f32->bf16 SWDGE cast-DMAs [128t, 2048i]; PE
    transposes (1 cyc/row) produce [j, t] bf16 tiles grouped in wide
    PSUM tiles; ACT casts psum->fp8 (hi), DVE subtracts psum - hi -> lo
    (mixed-dtype in, fp8 out).
  - psum[t, o] accumulates 32 DoubleRow matmuls (2 passes x 2 h x 8 b).
  - bias is pre-broadcast via a rank-1 f32r matmul into bbc; rowsum
    s[t] = sum_i bf16(x)[t, i] on Pool (consistent with hi+lo);
    epilogue fuses y = psum - s + bbc on DVE/Pool.
"""
import sys

sys.path.insert(0, "/opt/trn_rl_repo")
from contextlib import ExitStack

import numpy as np

import concourse.tile as tile
from concourse import bacc, mybir
from concourse.bass import ts
from concourse.bass_utils import run_bass_kernel_spmd
from concourse.masks import make_identity

F32 = mybir.dt.float32
F32R = mybir.dt.float32r
BF16 = mybir.dt.bfloat16
U16 = mybir.dt.uint16
F8 = mybir.dt.float8e4
P = 128

N_CORES = 8
B_DIM, S_DIM, I_DIM, O_DIM = 2, 2048, 4096, 4096
T_FULL = B_DIM * S_DIM          # 4096 tokens
T_SHARD = T_FULL // N_CORES     # 512 tokens per core
OUT_NAME = "y"


def build(T=T_SHARD, I=I_DIM, O=O_DIM, n_cores=N_CORES):
    H = I // 2048               # 2048-wide i-halves (j spans 16*128)
    TT = T // P                 # token tiles
    NB = 8                      # bits per byte
    K2 = I // 16                # u16 byte-pairs per weight row
    OSL = 512                   # o-slab width
    NSL = O // OSL

    nc = bacc.Bacc("TRN2", target_bir_lowering=False, debug=False,
                   num_devices=n_cores)
    x_d = nc.dram_tensor("x", [T, I], F32, kind="ExternalInput").ap()
    pw16_d = nc.dram_tensor("pw16", [O, K2], U16, kind="ExternalInput").ap()
    bias_d = nc.dram_tensor("bias", [O], F32, kind="ExternalInput").ap()
    y_d = nc.dram_tensor(OUT_NAME, [T, O], F32, kind="ExternalOutput").ap()

    with tile.TileContext(nc) as tc:
        with ExitStack() as ctx:
            const = ctx.enter_context(tc.tile_pool(name="const", bufs=1))
            persist = ctx.enter_context(tc.tile_pool(name="persist", bufs=1))
            stage = ctx.enter_context(tc.tile_pool(name="stage", bufs=1))

            ident_bf = const.tile([P, P], BF16)
            make_identity(nc, ident_bf[:])
            ones_r = const.tile([1, P], F32R)
            bias_r = const.tile([1, O], F32R)

            byteT2 = persist.tile([P, H, O], U16)
            # xT planes: [j, h, bh, tt, (b' p t)] fp8, 16KB/partition each
            xT_hi = persist.tile([P, H, 2, TT, 1024], F8)
            xT_lo = persist.tile([P, H, 2, TT, 1024], F8)

            xn_pool = ctx.enter_context(tc.tile_pool(name="xn", bufs=H * TT))
            pk_pool = ctx.enter_context(tc.tile_pool(name="pk", bufs=2))
            wt_pool = ctx.enter_context(tc.tile_pool(name="wt", bufs=2))
            scol_pool = ctx.enter_context(tc.tile_pool(name="scol", bufs=1))
            trash_pool = ctx.enter_context(tc.tile_pool(name="trash", bufs=2))
            bbc_pool = ctx.enter_context(tc.tile_pool(name="bbc", bufs=2))
            y_pool = ctx.enter_context(tc.tile_pool(name="ysb", bufs=4))
            ps_tr = ctx.enter_context(
                tc.tile_pool(name="ps_tr", bufs=2, space="PSUM"))
            ps_btr = ctx.enter_context(
                tc.tile_pool(name="ps_btr", bufs=2, space="PSUM"))
            ps_mm = ctx.enter_context(
                tc.tile_pool(name="ps_mm", bufs=3, space="PSUM"))
            ps_b_pool = ctx.enter_context(
                tc.tile_pool(name="ps_b", bufs=1, space="PSUM"))

            # pw quarters: 8 o-tiles (1024 o's) per regular DMA, u16 pairs
            OT = O // P
            QOT = min(8, OT)            # o-tiles per quarter
            NQ = OT // QOT
            pw_ap = pw16_d.rearrange("(ot p) k -> p ot k", p=P)
            pks = {}

            def pw_dma(q):
                pk = pk_pool.tile([P, QOT, K2], U16, name="pk", tag="pk")
                nc.gpsimd.dma_start(pk[:], pw_ap[:, ts(q, QOT), :])
                pks[q] = pk

            # PE u16 transposes of byte pairs + DVE evac into byteT2
            def pw_tr_evac(q):
                pk = pks.pop(q)
                for h in range(H):
                    for g in range(QOT // 4):
                        btr = ps_btr.tile([P, 4, P], BF16, name="btr", tag="btr")
                        for otl in range(4):
                            nc.tensor.transpose(
                                btr[:, otl, :],
                                pk[:, 4 * g + otl, ts(h, P)].bitcast(BF16),
                                ident_bf[:],
                            )
                        nc.vector.tensor_copy(
                            out=byteT2[:, h, q * QOT * P + 512 * g:
                                       q * QOT * P + 512 * (g + 1)],
                            in_=btr[:].rearrange("j a b -> j (a b)").bitcast(U16),
                        )

            # --- x chunks (SWDGE f32->bf16 cast DMAs) interleaved with pw ---
            xns = {}
            for tt in range(TT):
                for h in range(H):
                    xn = xn_pool.tile([P, P, 16], BF16, tag="xn")
                    nc.gpsimd.dma_start(
                        xn[:].rearrange("t j q -> t (j q)"),
                        x_d[ts(tt, P), ts(h, 2048)],
                    )
                    xns[h, tt] = xn
                if tt == 0:
                    pw_dma(0)
            for q in range(1, NQ):
                pw_dma(q)

            # --- bias/ones staging ---
            ones_f32 = stage.tile([1, P], F32)
            nc.vector.memset(ones_f32[:], 1.0)
            nc.vector.tensor_copy(out=ones_r[:], in_=ones_f32[:])
            bias_f32 = stage.tile([1, O], F32)
            nc.sync.dma_start(
                bias_f32[:], bias_d.rearrange("(b o) -> b o", b=1))
            nc.vector.tensor_copy(out=bias_r[:], in_=bias_f32[:])

            # --- unpack weight slabs 0,1 up front (DVE, 4x mode) ---
            def unpack(sl, wt):
                for h in range(H):
                    for b in range(NB):
                        if b < 7:
                            nc.vector.tensor_scalar(
                                out=wt[:, h, b, :],
                                in0=byteT2[:, h, ts(sl, OSL)],
                                scalar1=6 - b, scalar2=0x4040,
                                op0=mybir.AluOpType.logical_shift_left,
                                op1=mybir.AluOpType.bitwise_and,
                            )
                        else:
                            nc.vector.tensor_scalar(
                                out=wt[:, h, b, :],
                                in0=byteT2[:, h, ts(sl, OSL)],
                                scalar1=1, scalar2=0x4040,
                                op0=mybir.AluOpType.logical_shift_right,
                                op1=mybir.AluOpType.bitwise_and,
                            )

            wts = {}

            def alloc_unpack(sl):
                wts[sl] = wt_pool.tile([P, H, NB, OSL], U16, name="wt",
                                       tag="wt")
                unpack(sl, wts[sl])

            def mm_psum(sl, tt, wt, bbc):
                ps = ps_mm.tile([P, OSL], F32, name="ps", tag="mm")
                n = 0
                for plane in (xT_hi, xT_lo):
                    for h in range(H):
                        for bh in range(2):
                            for bp in range(4):
                                rhs = wt[:, h, 4 * bh + bp, :].bitcast(
                                    F8).rearrange("j (o p) -> j p o", p=2)
                                nc.tensor.matmul(
                                    ps[:],
                                    plane[:, h, bh, tt, ts(bp, 256)]
                                    .rearrange("j (p t) -> j p t", p=2),
                                    rhs,
                                    start=(n == 0), stop=(n == 31),
                                    perf_mode=mybir.MatmulPerfMode.DoubleRow,
                                )
                                n += 1
                y_sb = y_pool.tile([P, OSL], F32, name="y_sb", tag="y")
                nc.vector.scalar_tensor_tensor(
                    out=y_sb[:], in0=ps[:], scalar=s_col[:, tt:tt + 1],
                    in1=bbc[:],
                    op0=mybir.AluOpType.subtract,
                    op1=mybir.AluOpType.add,
                )
                nc.sync.dma_start(y_d[ts(tt, P), ts(sl, OSL)], y_sb[:])

            # bias broadcast for slab 0 (needed by the interleaved sl0 psums)
            ps_bias0 = ps_b_pool.tile([P, OSL], F32, name="psb", tag="psb")
            nc.tensor.matmul(ps_bias0[:], ones_r[:], bias_r[:, ts(0, OSL)],
                             start=True, stop=True)
            bbc0 = bbc_pool.tile([P, OSL], F32, name="bbc", tag="bbc")
            nc.scalar.copy(out=bbc0[:], in_=ps_bias0[:])

            def splits(tt):
                for h in range(H):
                    for bh in range(2):
                        ps = ps_tr.tile([P, 1024], BF16, tag="tr")
                        for bp in range(4):
                            for p in range(2):
                                nc.tensor.transpose(
                                    ps[:, ts(bp * 2 + p, P)],
                                    xns[h, tt][:, :, 8 * p + 4 * bh + bp],
                                    ident_bf[:],
                                )
                        nc.scalar.copy(out=xT_hi[:, h, bh, tt, :], in_=ps[:])
                        nc.vector.tensor_tensor(
                            out=xT_lo[:, h, bh, tt, :], in0=ps[:],
                            in1=xT_hi[:, h, bh, tt, :],
                            op=mybir.AluOpType.subtract,
                        )
                    # rowsum partial via tensor_scalar accumulator (on Pool)
                    trash = trash_pool.tile([P, P, 16], BF16, tag="trash")
                    nc.vector.tensor_scalar(
                        out=trash[:], in0=xns[h, tt][:],
                        scalar1=1.0, scalar2=0.0,
                        op0=mybir.AluOpType.mult, op1=mybir.AluOpType.add,
                        accum_out=parts[:, tt, h:h + 1],
                    )
                    if h == H - 1:
                        nc.vector.tensor_tensor(
                            out=s_col[:, tt:tt + 1], in0=parts[:, tt, 0:1],
                            in1=parts[:, tt, 1:2], op=mybir.AluOpType.add,
                        )

            # --- startup: splits, pw transposes, first unpacks, and sl0's
            # --- matmuls interleaved per token tile
            parts = scol_pool.tile([P, TT, H], F32)
            s_col = scol_pool.tile([P, TT], F32)
            for tt in range(TT):
                splits(tt)
                if tt == 0:
                    pw_tr_evac(0)
                    alloc_unpack(0)
                    if NSL > 1:
                        alloc_unpack(1)
                else:
                    mm_psum(0, tt - 1, wts[0], bbc0)
            mm_psum(0, TT - 1, wts[0], bbc0)
            for q in range(1, NQ):
                pw_tr_evac(q)

            # --- main loop over remaining o-slabs ---
            wts.pop(0)
            if NSL > 2:
                alloc_unpack(2)
            for sl in range(1, NSL):
                wt = wts.pop(sl)
                ps_bias = ps_b_pool.tile([P, OSL], F32, name="psb", tag="psb")
                nc.tensor.matmul(
                    ps_bias[:], ones_r[:], bias_r[:, ts(sl, OSL)],
                    start=True, stop=True,
                )
                bbc = bbc_pool.tile([P, OSL], F32, name="bbc", tag="bbc")
                nc.scalar.copy(out=bbc[:], in_=ps_bias[:])

                for tt in range(TT):
                    mm_psum(sl, tt, wt, bbc)

                if sl + 2 < NSL:
                    alloc_unpack(sl + 2)

    nc.compile()
    return nc


_NC = None


def _get_nc():
    global _NC
    if _NC is None:
        _NC = build()
    return _NC


def run(x, packed_weight, bias, trace=False):
    x = np.ascontiguousarray(np.asarray(x, dtype=np.float32))
    pw = np.ascontiguousarray(np.asarray(packed_weight).astype(np.uint8))
    bias = np.ascontiguousarray(np.asarray(bias, dtype=np.float32))
    assert x.shape == (B_DIM, S_DIM, I_DIM)
    assert pw.shape == (O_DIM, I_DIM // 8)
    assert bias.shape == (O_DIM,)

    nc = _get_nc()
    xs = x.reshape(T_FULL, I_DIM)
    pw16 = pw.view(np.uint16)
    in_maps = [
        {
            "x": np.ascontiguousarray(xs[c * T_SHARD:(c + 1) * T_SHARD]),
            "pw16": pw16,
            "bias": bias,
        }
        for c in range(N_CORES)
    ]
    res = run_bass_kernel_spmd(nc, in_maps, list(range(N_CORES)), trace=trace)
    y = np.concatenate(
        [res.results[c][OUT_NAME] for c in range(N_CORES)], axis=0
    )
    return y.reshape(B_DIM, S_DIM, O_DIM), res


def kernel(x, packed_weight, bias):
    y, _ = run(x, packed_weight, bias, trace=False)
    return y


# revision 23
# speedup vs baseline: 1.0599x; 1.0244x over previous
"""BitPackedLinear Trainium2 kernel (8-core SPMD, token-sharded, fp8 DoubleRow).

y = x @ W.T + bias, W = unpack_bits(packed_weight) in {-1,+1}, shapes:
  x [2, 2048, 4096] f32, packed_weight [4096, 512] u8, bias [4096] f32.

Sharding: data-parallel over tokens (4096 tokens -> 512/core). Each core
computes y_c = x_c @ W.T + bias for its token shard against the full
weight; the host just concatenates shards.

Device algorithm per core (matmuls are fp8e4 DoubleRow at 0.5 cyc/row,
one instruction contracts TWO 128-deep k-tiles):
  - x is split as x_bf = hi + lo with hi = e4m3(x_bf), lo = x_bf - hi
    (lo is exactly representable in e4m3, so hi+lo == bf16(x) exactly;
    end-to-end rel err ~1.7e-3, dominated by the bf16 load cast).
  - i-tiling: i = 2048h + 16j + 8p + b with j the PE partition, (p, b)
    byte-parity/bit, h the 2048-halves. The DoubleRow k-tile pair is p.
  - weights: pw is viewed as u16 [4096, 256] (byte pairs along k) and
    XBAR-transpose-DMA'd to byteT2 [128j, h, o]; each u16 holds bytes
    (2j, 2j+1) of one o. A single u16 shift+mask (<<(6-b), & 0x4040)
    then yields the e4m3 BIT PATTERN of {0, 2.0} (0x40) for BOTH
    parities of bit b at once -> one DVE op per (h, b, o-slab), all in
    the DVE 4x perf mode. The u16 result bitcast to fp8 is the moving
    [j, p, o] operand.
  - x chunks arrive as f32->bf16 SWDGE cast-DMAs [128t, 2048i]; PE
    transposes (1 cyc/row) produce [j, t] bf16 tiles grouped in wide
    PSUM tiles; ACT casts psum->fp8 (hi), DVE subtracts psum - hi -> lo
    (mixed-dtype in, fp8 out).
  - psum[t, o] accumulates 32 DoubleRow matmuls (2 passes x 2 h x 8 b).
  - bias is pre-broadcast via a rank-1 f32r matmul into bbc; rowsum
    s[t] = sum_i bf16(x)[t, i] on Pool (consistent with hi+lo);
    epilogue fuses y = psum - s + bbc on DVE/Pool.
"""
import sys

sys.path.insert(0, "/opt/trn_rl_repo")
from contextlib import ExitStack

import numpy as np

import concourse.tile as tile
from concourse import bacc, mybir
from concourse.bass import ts
from concourse.bass_utils import run_bass_kernel_spmd
from concourse.masks import make_identity

F32 = mybir.dt.float32
F32R = mybir.dt.float32r
BF16 = mybir.dt.bfloat16
U16 = mybir.dt.uint16
F8 = mybir.dt.float8e4
P = 128

N_CORES = 8
B_DIM, S_DIM, I_DIM, O_DIM = 2, 2048, 4096, 4096
T_FULL = B_DIM * S_DIM          # 4096 tokens
T_SHARD = T_FULL // N_CORES     # 512 tokens per core
OUT_NAME = "y"


def build(T=T_SHARD, I=I_DIM, O=O_DIM, n_cores=N_CORES):
    H = I // 2048               # 2048-wide i-halves (j spans 16*128)
    TT = T // P                 # token tiles
    NB = 8                      # bits per byte
    K2 = I // 16                # u16 byte-pairs per weight row
    OSL = 512                   # o-slab width
    NSL = O // OSL

    nc = bacc.Bacc("TRN2", target_bir_lowering=False, debug=False,
                   num_devices=n_cores)
    x_d = nc.dram_tensor("x", [T, I], F32, kind="ExternalInput").ap()
    pw16_d = nc.dram_tensor("pw16", [O, K2], U16, kind="ExternalInput").ap()
    bias_d = nc.dram_tensor("bias", [O], F32, kind="ExternalInput").ap()
    y_d = nc.dram_tensor(OUT_NAME, [T, O], F32, kind="ExternalOutput").ap()

    with tile.TileContext(nc) as tc:
        with ExitStack() as ctx:
            const = ctx.enter_context(tc.tile_pool(name="const", bufs=1))
            persist = ctx.enter_context(tc.tile_pool(name="persist", bufs=1))
            stage = ctx.enter_context(tc.tile_pool(name="stage", bufs=1))

            ident_bf = const.tile([P, P], BF16)
            make_identity(nc, ident_bf[:])

            byteT2 = persist.tile([P, H, O], U16)
            # xT planes: [j, h, bh, tt, (b' p t)] fp8, 16KB/partition each
            xT_hi = persist.tile([P, H, 2, TT, 1024], F8)
            xT_lo = persist.tile([P, H, 2, TT, 1024], F8)

            xn_pool = ctx.enter_context(tc.tile_pool(name="xn", bufs=H * TT))
            pk_pool = ctx.enter_context(tc.tile_pool(name="pk", bufs=2))
            wt_pool = ctx.enter_context(tc.tile_pool(name="wt", bufs=2))
            scol_pool = ctx.enter_context(tc.tile_pool(name="scol", bufs=1))
            trash_pool = ctx.enter_context(tc.tile_pool(name="trash", bufs=2))
            y_pool = ctx.enter_context(tc.tile_pool(name="ysb", bufs=4))
            ps_tr = ctx.enter_context(
                tc.tile_pool(name="ps_tr", bufs=2, space="PSUM"))
            ps_btr = ctx.enter_context(
                tc.tile_pool(name="ps_btr", bufs=2, space="PSUM"))
            ps_mm = ctx.enter_context(
                tc.tile_pool(name="ps_mm", bufs=4, space="PSUM"))

            # pw quarters: 8 o-tiles (1024 o's) per regular DMA, u16 pairs
            OT = O // P
            QOT = min(8, OT)            # o-tiles per quarter
            NQ = OT // QOT
            pw_ap = pw16_d.rearrange("(ot p) k -> p ot k", p=P)
            pks = {}

            def pw_dma(q):
                pk = pk_pool.tile([P, QOT, K2], U16, name="pk", tag="pk")
                nc.gpsimd.dma_start(pk[:], pw_ap[:, ts(q, QOT), :])
                pks[q] = pk

            # PE u16 transposes of byte pairs + DVE evac into byteT2
            def pw_tr_evac(q):
                pk = pks.pop(q)
                for h in range(H):
                    for g in range(QOT // 4):
                        btr = ps_btr.tile([P, 4, P], BF16, name="btr", tag="btr")
                        for otl in range(4):
                            nc.tensor.transpose(
                                btr[:, otl, :],
                                pk[:, 4 * g + otl, ts(h, P)].bitcast(BF16),
                                ident_bf[:],
                            )
                        nc.vector.tensor_copy(
                            out=byteT2[:, h, q * QOT * P + 512 * g:
                                       q * QOT * P + 512 * (g + 1)],
                            in_=btr[:].rearrange("j a b -> j (a b)").bitcast(U16),
                        )

            # --- x chunks (SWDGE f32->bf16 cast DMAs) interleaved with pw ---
            xns = {}
            for tt in range(TT):
                for h in range(H):
                    xn = xn_pool.tile([P, P, 16], BF16, tag="xn")
                    nc.gpsimd.dma_start(
                        xn[:].rearrange("t j q -> t (j q)"),
                        x_d[ts(tt, P), ts(h, 2048)],
                    )
                    xns[h, tt] = xn
                if tt == 0:
                    pw_dma(0)
            for q in range(1, NQ):
                pw_dma(q)

            # --- bias: DMA the row, broadcast to all partitions (Pool) ---
            bias_f32 = stage.tile([1, O], F32)
            nc.sync.dma_start(
                bias_f32[:], bias_d.rearrange("(b o) -> b o", b=1))
            bbc_all = persist.tile([P, O], F32)
            nc.gpsimd.partition_broadcast(bbc_all[:], bias_f32[:])

            # --- unpack weight slabs 0,1 up front (DVE, 4x mode) ---
            def unpack(sl, wt):
                for h in range(H):
                    for b in range(NB):
                        if b < 7:
                            nc.vector.tensor_scalar(
                                out=wt[:, h, b, :],
                                in0=byteT2[:, h, ts(sl, OSL)],
                                scalar1=6 - b, scalar2=0x4040,
                                op0=mybir.AluOpType.logical_shift_left,
                                op1=mybir.AluOpType.bitwise_and,
                            )
                        else:
                            nc.vector.tensor_scalar(
                                out=wt[:, h, b, :],
                                in0=byteT2[:, h, ts(sl, OSL)],
                                scalar1=1, scalar2=0x4040,
                                op0=mybir.AluOpType.logical_shift_right,
                                op1=mybir.AluOpType.bitwise_and,
                            )

            wts = {}

            def alloc_unpack(sl):
                wts[sl] = wt_pool.tile([P, H, NB, OSL], U16, name="wt",
                                       tag="wt")
                unpack(sl, wts[sl])

            def mm_psum(sl, tt, wt, olo=0, ow=OSL):
                ps = ps_mm.tile([P, OSL], F32, name="ps", tag="mm")
                n = 0
                for plane in (xT_hi, xT_lo):
                    for h in range(H):
                        for bh in range(2):
                            for bp in range(4):
                                rhs = wt[:, h, 4 * bh + bp,
                                         olo:olo + ow].bitcast(
                                    F8).rearrange("j (o p) -> j p o", p=2)
                                nc.tensor.matmul(
                                    ps[:, :ow],
                                    plane[:, h, bh, tt, ts(bp, 256)]
                                    .rearrange("j (p t) -> j p t", p=2),
                                    rhs,
                                    start=(n == 0), stop=(n == 31),
                                    perf_mode=mybir.MatmulPerfMode.DoubleRow,
                                )
                                n += 1
                y_sb = y_pool.tile([P, OSL], F32, name="y_sb", tag="y")
                nc.vector.scalar_tensor_tensor(
                    out=y_sb[:, :ow], in0=ps[:, :ow],
                    scalar=s_col[:, tt:tt + 1],
                    in1=bbc_all[:, sl * OSL + olo:sl * OSL + olo + ow],
                    op0=mybir.AluOpType.subtract,
                    op1=mybir.AluOpType.add,
                )
                nc.sync.dma_start(
                    y_d[ts(tt, P), sl * OSL + olo:sl * OSL + olo + ow],
                    y_sb[:, :ow])

            def splits(tt):
                for h in range(H):
                    for bh in range(2):
                        ps = ps_tr.tile([P, 1024], BF16, tag="tr")
                        for bp in range(4):
                            for p in range(2):
                                nc.tensor.transpose(
                                    ps[:, ts(bp * 2 + p, P)],
                                    xns[h, tt][:, :, 8 * p + 4 * bh + bp],
                                    ident_bf[:],
                                )
                        nc.scalar.copy(out=xT_hi[:, h, bh, tt, :], in_=ps[:])
                        nc.vector.tensor_tensor(
                            out=xT_lo[:, h, bh, tt, :], in0=ps[:],
                            in1=xT_hi[:, h, bh, tt, :],
                            op=mybir.AluOpType.subtract,
                        )
                    # rowsum partial via tensor_scalar accumulator (on Pool)
                    trash = trash_pool.tile([P, P, 16], BF16, tag="trash")
                    nc.vector.tensor_scalar(
                        out=trash[:], in0=xns[h, tt][:],
                        scalar1=1.0, scalar2=0.0,
                        op0=mybir.AluOpType.mult, op1=mybir.AluOpType.add,
                        accum_out=parts[:, tt, h:h + 1],
                    )
                    if h == H - 1:
                        nc.vector.tensor_tensor(
                            out=s_col[:, tt:tt + 1], in0=parts[:, tt, 0:1],
                            in1=parts[:, tt, 1:2], op=mybir.AluOpType.add,
                        )

            # --- startup: splits, pw transposes, first unpacks, and sl0's
            # --- matmuls interleaved per token tile
            parts = scol_pool.tile([P, TT, H], F32)
            s_col = scol_pool.tile([P, TT], F32)
            for tt in range(TT):
                splits(tt)
                if tt == 0:
                    pw_tr_evac(0)
                    alloc_unpack(0)
                    if NSL > 1:
                        alloc_unpack(1)
                else:
                    mm_psum(0, tt - 1, wts[0])
            mm_psum(0, TT - 1, wts[0])
            for q in range(1, NQ):
                pw_tr_evac(q)

            # --- main loop over remaining o-slabs ---
            wts.pop(0)
            if NSL > 2:
                alloc_unpack(2)
            for sl in range(1, NSL):
                wt = wts.pop(sl)
                for tt in range(TT):
                    if sl == NSL - 1 and tt == TT - 1:
                        # split the final psum: shorter drain tail
                        mm_psum(sl, tt, wt, 0, OSL // 2)
                        mm_psum(sl, tt, wt, OSL // 2, OSL // 2)
                    else:
                        mm_psum(sl, tt, wt)

                if sl + 2 < NSL:
                    alloc_unpack(sl + 2)

    nc.compile()
    return nc


_NC = None


def _get_nc():
    global _NC
    if _NC is None:
        _NC = build()
    return _NC


def run(x, packed_weight, bias, trace=False):
    x = np.ascontiguousarray(np.asarray(x, dtype=np.float32))
    pw = np.ascontiguousarray(np.asarray(packed_weight).astype(np.uint8))
    bias = np.ascontiguousarray(np.asarray(bias, dtype=np.float32))
    assert x.shape == (B_DIM, S_DIM, I_DIM)
    assert pw.shape == (O_DIM, I_DIM // 8)
    assert bias.shape == (O_DIM,)

    nc = _get_nc()
    xs = x.reshape(T_FULL, I_DIM)
    pw16 = pw.view(np.uint16)
    in_maps = [
        {
            "x": np.ascontiguousarray(xs[c * T_SHARD:(c + 1) * T_SHARD]),
            "pw16": pw16,
            "bias": bias,
        }
        for c in range(N_CORES)
    ]
    res = run_bass_kernel_spmd(nc, in_maps, list(range(N_CORES)), trace=trace)
    y = np.concatenate(
        [res.results[c][OUT_NAME] for c in range(N_CORES)], axis=0
    )
    return y.reshape(B_DIM, S_DIM, O_DIM), res


def kernel(x, packed_weight, bias):
    y, _ = run(x, packed_weight, bias, trace=False)
    return y


# revision 24
# speedup vs baseline: 1.0785x; 1.0175x over previous
"""BitPackedLinear Trainium2 kernel (8-core SPMD, token-sharded, fp8 DoubleRow).

y = x @ W.T + bias, W = unpack_bits(packed_weight) in {-1,+1}, shapes:
  x [2, 2048, 4096] f32, packed_weight [4096, 512] u8, bias [4096] f32.

Sharding: data-parallel over tokens (4096 tokens -> 512/core). Each core
computes y_c = x_c @ W.T + bias for its token shard against the full
weight; the host just concatenates shards.

Device algorithm per core (matmuls are fp8e4 DoubleRow at 0.5 cyc/row,
one instruction contracts TWO 128-deep k-tiles):
  - x is split as x_bf = hi + lo with hi = e4m3(x_bf), lo = x_bf - hi
    (lo is exactly representable in e4m3, so hi+lo == bf16(x) exactly;
    end-to-end rel err ~1.7e-3, dominated by the bf16 load cast).
  - i-tiling: i = 2048h + 16j + 8p + b with j the PE partition, (p, b)
    byte-parity/bit, h the 2048-halves. The DoubleRow k-tile pair is p.
  - weights: pw is viewed as u16 [4096, 256] (byte pairs along k) and
    XBAR-transpose-DMA'd to byteT2 [128j, h, o]; each u16 holds bytes
    (2j, 2j+1) of one o. A single u16 shift+mask (<<(6-b), & 0x4040)
    then yields the e4m3 BIT PATTERN of {0, 2.0} (0x40) for BOTH
    parities of bit b at once -> one DVE op per (h, b, o-slab), all in
    the DVE 4x perf mode. The u16 result bitcast to fp8 is the moving
    [j, p, o] operand.
  - x chunks arrive as f32->bf16 SWDGE cast-DMAs [128t, 2048i]; PE
    transposes (1 cyc/row) produce [j, t] bf16 tiles grouped in wide
    PSUM tiles; ACT casts psum->fp8 (hi), DVE subtracts psum - hi -> lo
    (mixed-dtype in, fp8 out).
  - psum[t, o] accumulates 32 DoubleRow matmuls (2 passes x 2 h x 8 b).
  - bias is pre-broadcast via a rank-1 f32r matmul into bbc; rowsum
    s[t] = sum_i bf16(x)[t, i] on Pool (consistent with hi+lo);
    epilogue fuses y = psum - s + bbc on DVE/Pool.
"""
import sys

sys.path.insert(0, "/opt/trn_rl_repo")
from contextlib import ExitStack

import numpy as np

import concourse.tile as tile
from concourse import bacc, mybir
from concourse.bass import ts
from concourse.bass_utils import run_bass_kernel_spmd
from concourse.masks import make_identity

F32 = mybir.dt.float32
F32R = mybir.dt.float32r
BF16 = mybir.dt.bfloat16
U16 = mybir.dt.uint16
F8 = mybir.dt.float8e4
P = 128

N_CORES = 8
B_DIM, S_DIM, I_DIM, O_DIM = 2, 2048, 4096, 4096
T_FULL = B_DIM * S_DIM          # 4096 tokens
T_SHARD = T_FULL // N_CORES     # 512 tokens per core
OUT_NAME = "y"


def build(T=T_SHARD, I=I_DIM, O=O_DIM, n_cores=N_CORES):
    H = I // 2048               # 2048-wide i-halves (j spans 16*128)
    TT = T // P                 # token tiles
    NB = 8                      # bits per byte
    K2 = I // 16                # u16 byte-pairs per weight row
    OSL = 512                   # o-slab width
    NSL = O // OSL

    nc = bacc.Bacc("TRN2", target_bir_lowering=False, debug=False,
                   num_devices=n_cores)
    x_d = nc.dram_tensor("x", [T, I], F32, kind="ExternalInput").ap()
    pw16_d = nc.dram_tensor("pw16", [O, K2], U16, kind="ExternalInput").ap()
    bias_d = nc.dram_tensor("bias", [O], F32, kind="ExternalInput").ap()
    y_d = nc.dram_tensor(OUT_NAME, [T, O], F32, kind="ExternalOutput").ap()

    with tile.TileContext(nc) as tc:
        with ExitStack() as ctx:
            const = ctx.enter_context(tc.tile_pool(name="const", bufs=1))
            persist = ctx.enter_context(tc.tile_pool(name="persist", bufs=1))
            stage = ctx.enter_context(tc.tile_pool(name="stage", bufs=1))

            ident_bf = const.tile([P, P], BF16)
            make_identity(nc, ident_bf[:])

            byteT2 = persist.tile([P, H, O], U16)
            # xT planes: [j, h, bh, tt, (b' p t)] fp8, 16KB/partition each
            xT_hi = persist.tile([P, H, 2, TT, 1024], F8)
            xT_lo = persist.tile([P, H, 2, TT, 1024], F8)

            xn_pool = ctx.enter_context(tc.tile_pool(name="xn", bufs=H * TT))
            pk_pool = ctx.enter_context(tc.tile_pool(name="pk", bufs=2))
            wt_pool = ctx.enter_context(tc.tile_pool(name="wt", bufs=2))
            scol_pool = ctx.enter_context(tc.tile_pool(name="scol", bufs=1))
            trash_pool = ctx.enter_context(tc.tile_pool(name="trash", bufs=2))
            y_pool = ctx.enter_context(tc.tile_pool(name="ysb", bufs=4))
            ps_tr = ctx.enter_context(
                tc.tile_pool(name="ps_tr", bufs=3, space="PSUM"))
            ps_btr = ctx.enter_context(
                tc.tile_pool(name="ps_btr", bufs=1, space="PSUM"))
            ps_mm = ctx.enter_context(
                tc.tile_pool(name="ps_mm", bufs=4, space="PSUM"))

            # pw quarters: 8 o-tiles (1024 o's) per regular DMA, u16 pairs
            OT = O // P
            QOT = min(8, OT)            # o-tiles per quarter
            NQ = OT // QOT
            pw_ap = pw16_d.rearrange("(ot p) k -> p ot k", p=P)
            pks = {}

            def pw_dma(q):
                pk = pk_pool.tile([P, QOT, K2], U16, name="pk", tag="pk")
                nc.gpsimd.dma_start(pk[:], pw_ap[:, ts(q, QOT), :])
                pks[q] = pk

            # PE u16 transposes of byte pairs + DVE evac into byteT2
            def pw_tr_evac(q):
                pk = pks.pop(q)
                for h in range(H):
                    for g in range(QOT // 4):
                        btr = ps_btr.tile([P, 4, P], BF16, name="btr", tag="btr")
                        for otl in range(4):
                            nc.tensor.transpose(
                                btr[:, otl, :],
                                pk[:, 4 * g + otl, ts(h, P)].bitcast(BF16),
                                ident_bf[:],
                            )
                        nc.vector.tensor_copy(
                            out=byteT2[:, h, q * QOT * P + 512 * g:
                                       q * QOT * P + 512 * (g + 1)],
                            in_=btr[:].rearrange("j a b -> j (a b)").bitcast(U16),
                        )

            # --- x chunks (SWDGE f32->bf16 cast DMAs) interleaved with pw ---
            xns = {}
            for tt in range(TT):
                for h in range(H):
                    xn = xn_pool.tile([P, P, 16], BF16, tag="xn")
                    nc.gpsimd.dma_start(
                        xn[:].rearrange("t j q -> t (j q)"),
                        x_d[ts(tt, P), ts(h, 2048)],
                    )
                    xns[h, tt] = xn
                if tt == 0:
                    pw_dma(0)
            for q in range(1, NQ):
                pw_dma(q)

            # --- bias: DMA the row, broadcast to all partitions (Pool) ---
            bias_f32 = stage.tile([1, O], F32)
            nc.sync.dma_start(
                bias_f32[:], bias_d.rearrange("(b o) -> b o", b=1))
            bbc_all = persist.tile([P, O], F32)
            nc.gpsimd.partition_broadcast(bbc_all[:], bias_f32[:])

            # --- unpack weight slabs 0,1 up front (DVE, 4x mode) ---
            def unpack(sl, wt):
                for h in range(H):
                    for b in range(NB):
                        if b < 7:
                            nc.vector.tensor_scalar(
                                out=wt[:, h, b, :],
                                in0=byteT2[:, h, ts(sl, OSL)],
                                scalar1=6 - b, scalar2=0x4040,
                                op0=mybir.AluOpType.logical_shift_left,
                                op1=mybir.AluOpType.bitwise_and,
                            )
                        else:
                            nc.vector.tensor_scalar(
                                out=wt[:, h, b, :],
                                in0=byteT2[:, h, ts(sl, OSL)],
                                scalar1=1, scalar2=0x4040,
                                op0=mybir.AluOpType.logical_shift_right,
                                op1=mybir.AluOpType.bitwise_and,
                            )

            wts = {}

            def alloc_unpack(sl):
                wts[sl] = wt_pool.tile([P, H, NB, OSL], U16, name="wt",
                                       tag="wt")
                unpack(sl, wts[sl])

            def mm_psum(sl, tt, wt, olo=0, ow=OSL):
                ps = ps_mm.tile([P, OSL], F32, name="ps", tag="mm")
                n = 0
                for plane in (xT_hi, xT_lo):
                    for h in range(H):
                        for bh in range(2):
                            for bp in range(4):
                                rhs = wt[:, h, 4 * bh + bp,
                                         olo:olo + ow].bitcast(
                                    F8).rearrange("j (o p) -> j p o", p=2)
                                nc.tensor.matmul(
                                    ps[:, :ow],
                                    plane[:, h, bh, tt, ts(bp, 256)]
                                    .rearrange("j (p t) -> j p t", p=2),
                                    rhs,
                                    start=(n == 0), stop=(n == 31),
                                    perf_mode=mybir.MatmulPerfMode.DoubleRow,
                                )
                                n += 1
                y_sb = y_pool.tile([P, OSL], F32, name="y_sb", tag="y")
                nc.vector.scalar_tensor_tensor(
                    out=y_sb[:, :ow], in0=ps[:, :ow],
                    scalar=s_col[:, tt:tt + 1],
                    in1=bbc_all[:, sl * OSL + olo:sl * OSL + olo + ow],
                    op0=mybir.AluOpType.subtract,
                    op1=mybir.AluOpType.add,
                )
                nc.sync.dma_start(
                    y_d[ts(tt, P), sl * OSL + olo:sl * OSL + olo + ow],
                    y_sb[:, :ow])

            def splits(tt):
                for h in range(H):
                    for bh in range(2):
                        ps = ps_tr.tile([P, 1024], BF16, tag="tr")
                        for bp in range(4):
                            for p in range(2):
                                nc.tensor.transpose(
                                    ps[:, ts(bp * 2 + p, P)],
                                    xns[h, tt][:, :, 8 * p + 4 * bh + bp],
                                    ident_bf[:],
                                )
                        nc.scalar.copy(out=xT_hi[:, h, bh, tt, :], in_=ps[:])
                        nc.vector.tensor_tensor(
                            out=xT_lo[:, h, bh, tt, :], in0=ps[:],
                            in1=xT_hi[:, h, bh, tt, :],
                            op=mybir.AluOpType.subtract,
                        )
                    # rowsum partial via tensor_scalar accumulator (on Pool)
                    trash = trash_pool.tile([P, P, 16], BF16, tag="trash")
                    nc.vector.tensor_scalar(
                        out=trash[:], in0=xns[h, tt][:],
                        scalar1=1.0, scalar2=0.0,
                        op0=mybir.AluOpType.mult, op1=mybir.AluOpType.add,
                        accum_out=parts[:, tt, h:h + 1],
                    )
                    if h == H - 1:
                        nc.vector.tensor_tensor(
                            out=s_col[:, tt:tt + 1], in0=parts[:, tt, 0:1],
                            in1=parts[:, tt, 1:2], op=mybir.AluOpType.add,
                        )

            # --- startup: splits, pw transposes, first unpacks, and sl0's
            # --- matmuls interleaved per token tile
            parts = scol_pool.tile([P, TT, H], F32)
            s_col = scol_pool.tile([P, TT], F32)
            for tt in range(TT):
                splits(tt)
                if tt == 0:
                    pw_tr_evac(0)
                    alloc_unpack(0)
                else:
                    if tt == 1 and NSL > 1:
                        alloc_unpack(1)
                    mm_psum(0, tt - 1, wts[0])
            if NSL > 1 and 1 not in wts:
                alloc_unpack(1)
            mm_psum(0, TT - 1, wts[0])
            for q in range(1, NQ):
                pw_tr_evac(q)

            # --- main loop over remaining o-slabs ---
            wts.pop(0)
            if NSL > 2:
                alloc_unpack(2)
            for sl in range(1, NSL):
                wt = wts.pop(sl)
                for tt in range(TT):
                    if sl == NSL - 1 and tt == TT - 1:
                        # split the final psum: shorter drain tail
                        mm_psum(sl, tt, wt, 0, OSL // 2)
                        mm_psum(sl, tt, wt, OSL // 2, OSL // 2)
                    else:
                        mm_psum(sl, tt, wt)

                if sl + 2 < NSL:
                    alloc_unpack(sl + 2)

    nc.compile()
    return nc


_NC = None


def _get_nc():
    global _NC
    if _NC is None:
        _NC = build()
    return _NC


def run(x, packed_weight, bias, trace=False):
    x = np.ascontiguousarray(np.asarray(x, dtype=np.float32))
    pw = np.ascontiguousarray(np.asarray(packed_weight).astype(np.uint8))
    bias = np.ascontiguousarray(np.asarray(bias, dtype=np.float32))
    assert x.shape == (B_DIM, S_DIM, I_DIM)
    assert pw.shape == (O_DIM, I_DIM // 8)
    assert bias.shape == (O_DIM,)

    nc = _get_nc()
    xs = x.reshape(T_FULL, I_DIM)
    pw16 = pw.view(np.uint16)
    in_maps = [
        {
            "x": np.ascontiguousarray(xs[c * T_SHARD:(c + 1) * T_SHARD]),
            "pw16": pw16,
            "bias": bias,
        }
        for c in range(N_CORES)
    ]
    res = run_bass_kernel_spmd(nc, in_maps, list(range(N_CORES)), trace=trace)
    y = np.concatenate(
        [res.results[c][OUT_NAME] for c in range(N_CORES)], axis=0
    )
    return y.reshape(B_DIM, S_DIM, O_DIM), res


def kernel(x, packed_weight, bias):
    y, _ = run(x, packed_weight, bias, trace=False)
    return y
